# revision 49
# baseline (speedup 1.0000x reference)
"""MiniBatchDiscrimination Trainium2 kernel.

Math (per reference):
    act = (x @ W).reshape(B, K, D)              # B=256, K=100, D=50
    l1[i,k,j] = sum_d |act[i,k,d] - act[j,k,d]|
    features[i,k] = sum_j exp(-l1[i,k,j])
    out = concat([x, features], axis=1)

Sharding: kernels K are sharded across the 8 cores -- 13 kernels (650
columns of act_T) per core, K padded 100->104 with zero weight columns.
No collectives; each core handles the full BxB pairwise work for its 13
kernels.

Per-core algorithm (transposed layout, act_T[col, j] with col=(k,d)):
  Phase A: act_T = W_shard.T @ x.T on PE, cast to bf16 (plus an fp32
    upcast used as the tensor_scalar per-partition operand).  A small
    row-sum table A[r, j] = sum_d act[(r,d), j] is also computed on PE,
    and a bias table A_bias[32b+r, 2g+h] = -A[r, 8g+2b+h] is built with
    8 strided SBUF->SBUF DMAs.
  Phase B uses |y| = 2*relu(y) - y, i.e.
    l1[r; i, j] = 2*sum_d relu(act_j - act_i) - A[r, j] + A[r, i].
    - diff rows split 650 = 5*128 + 10 into 5 full chunks + a sliver.
      Per (i, chunk): one op computing relu(act_T[:, j-window] - act_T[:, i]):
      chunk 0 on ScalarE (Relu activation, per-partition bias), chunks
      1-4 on DVE tensor_scalar (bf16 4x mode), sliver on GpSimd.
    - PE d-reduction: block-diagonal 2.0-valued stationary contracts each
      chunk into PSUM; the sliver chunk's tile carries 13 extra constant
      rows holding A (bf16) so the same matmul also subtracts A[r, j]
      (stationary entries -1) -- no separate A-correction matmuls.
    - ScalarE exp: out = Exp(-pl1 + bias) with bias = -A[r, i] from the
      A_bias table, so the result is the true E[i,j] = exp(-l1); the
      accum_out gives sum_j for free.  Diagonal i==j is exactly 1.
  Symmetry skew (E[i,j] = E[j,i] up to fp32 rounding): i-block q of
    NBLK computes only j >= q*BLK.  The skipped contributions
    features[i] += sum_{j < block(i)} E[i,j] = sum_{i' in lower blocks}
    E[i', j=i] are recovered by accumulating PE column-sum matmuls
    (ones stationary over the (b,r) partition packing) of the retained
    exp tiles into a [13, 256-BLK] PSUM tile, shipped out as 'tsum'.
  Host: features = accum feats + tsum completion; concat with x.
"""

import numpy as np
import ml_dtypes
from contextlib import ExitStack

import concourse.bass as bass
import concourse.bacc as bacc
import concourse.tile as tile
from concourse import mybir
from concourse.bass_utils import run_bass_kernel_spmd

B = 256          # batch
IN_D = 1024      # input dim
NK = 13          # kernels per core (8*13 = 104 >= 100)
DK = 50          # dim per kernel
COLS = NK * DK   # 650 act_T rows per core
N_CORES = 8
NBLK = 8         # symmetry skew blocks over the batch
BLK = B // NBLK  # i/j block size
GPB = 32 // NBLK     # g-octets per block
FULL = [(0, 128), (128, 128), (256, 128), (384, 128), (512, 128)]
SLIV = (640, 10)

F32 = mybir.dt.float32
BF16 = mybir.dt.bfloat16


def build_nc():
    nc = bacc.Bacc()
    xT_d = nc.declare_dram_parameter("xT", [IN_D, B], BF16, isOutput=False)
    w_d = nc.declare_dram_parameter("w", [IN_D, COLS], BF16, isOutput=False)
    s_d = nc.declare_dram_parameter("s", [128, 416], BF16, isOutput=False)
    feat_d = nc.declare_dram_parameter("feat", [128, 64], F32, isOutput=True)
    tsum_d = nc.declare_dram_parameter("tsum", [NK, B - BLK], F32, isOutput=True)

    with ExitStack() as ctx:
        tc = ctx.enter_context(tile.TileContext(nc))
        const_pool = ctx.enter_context(tc.tile_pool(name="const", bufs=1))
        psum_a = ctx.enter_context(tc.tile_pool(name="psum_a", bufs=2, space="PSUM"))
        psum_b = ctx.enter_context(tc.tile_pool(name="psum_b", bufs=4, space="PSUM"))
        diff_pool = ctx.enter_context(tc.tile_pool(name="diff", bufs=8))
        junk_pool = ctx.enter_context(tc.tile_pool(name="junk", bufs=4))

        # ---- load inputs (batched DMAs, ordered by first use) ----
        s_tile = const_pool.tile([128, 416], BF16, tag="s")
        nc.sync.dma_start(out=s_tile[:], in_=s_d[:])
        xt_all = const_pool.tile([128, 8 * B], BF16, tag="xt")
        nc.sync.dma_start(
            out=xt_all[:].rearrange("p (k j) -> p k j", k=8),
            in_=xT_d[:].rearrange("(k p) j -> p k j", k=8),
        )
        # w loaded in 3 slices so early chunks' matmuls start before the
        # whole tensor lands (512B+ descriptor runs keep DMA cost low)
        w_all = const_pool.tile([128, 8 * COLS], BF16, tag="w")
        w_view = w_all[:].rearrange("p (k c) -> p k c", k=8)
        for c0, c1 in ((0, 256), (256, 512), (512, 650)):
            nc.sync.dma_start(
                out=w_view[:, :, c0:c1],
                in_=w_d[:, c0:c1].rearrange("(k p) c -> p k c", k=8),
            )

        # ---- PE warm-up during the DMA wait (p-state ramp to full clock) ----
        wu = const_pool.tile([128, 384], BF16, tag="wu")
        nc.vector.memset(wu[:], 0.0)
        pwu = psum_b.tile([128, 512], F32, tag="pl1")
        for _ in range(6):
            nc.tensor.matmul(
                pwu[:, 0:256], wu[:, 256:384], wu[:, 0:256],
                start=True, stop=True,
            )

        # ---- Phase A: act_T = W.T @ xT  (per chunk of act_T rows) ----
        CHUNKS = FULL + [SLIV]
        act_bf = []   # bf16 streaming operand
        act_f32 = []  # fp32 upcast (tensor_scalar per-partition operand)
        for t, (mstart, msz) in enumerate(CHUNKS):
            pa = psum_a.tile([msz, B], F32, tag="pa")
            for k in range(8):
                nc.tensor.matmul(
                    pa[:],
                    w_all[:, COLS * k + mstart:COLS * k + mstart + msz],
                    xt_all[:, B * k:B * (k + 1)],
                    start=(k == 0),
                    stop=(k == 7),
                )
            tb = const_pool.tile([msz, B], BF16, tag=f"actb{t}")
            nc.vector.tensor_copy(tb[:], pa[:])
            act_bf.append(tb)
            if t == 0:
                # chunk 0 mostly runs on ScalarE (Relu bias): negation needed,
                # plus the fp32 upcast for the GpSimd-assigned (b,h) slot
                tn = const_pool.tile([msz, B], F32, tag="actn0")
                nc.gpsimd.tensor_scalar_mul(tn[:], tb[:], -1.0)
                act_neg0 = tn
                tf = const_pool.tile([msz, B], F32, tag="actf0")
                nc.gpsimd.tensor_copy(tf[:], tb[:])
                act_f32.append(tf)
            else:
                tf = const_pool.tile([msz, B], F32, tag=f"actf{t}")
                nc.gpsimd.tensor_copy(tf[:], tb[:])
                act_f32.append(tf)

        # A[r, j] = sum_d act[(r,d), j]  (exact fp32 accumulation of bf16)
        pA = psum_a.tile([32, B], F32, tag="pa")
        for t, (mstart, msz) in enumerate(CHUNKS):
            nc.tensor.matmul(
                pA[:],
                s_tile[0:msz, 192 + 32 * t:192 + 32 * t + 32],
                act_bf[t][:],
                start=(t == 0),
                stop=(t == len(CHUNKS) - 1),
            )
        # A-table prep runs on ScalarE (idle during phase A) so DVE/GpSimd
        # program order stays free to start phase-B diffs chunk by chunk.
        a_bf = const_pool.tile([NK, B], BF16, tag="a_bf")
        nc.scalar.copy(a_bf[:], pA[0:NK, :])
        a_neg = const_pool.tile([NK, B], F32, tag="a_neg")
        nc.scalar.mul(a_neg[:], a_bf[:], -1.0)

        # bias table: A_bias[32b + r, 2g + h] = -A[r, 8g + 2b + h]
        # (zeroed first: rows 32b+13..32b+31 feed exp on don't-care partitions
        # and must stay finite for the completion matmul's 0-weight contract)
        a_bias = const_pool.tile([128, 64], F32, tag="a_bias")
        nc.vector.memset(a_bias[:], 0.0)
        a_neg_v = a_neg[:].rearrange("r (g b h) -> r g b h", b=4, h=2)
        for b in range(4):
            # [13 parts, (g,h)=64] per b-group (HWDGE is idle by this point)
            nc.sync.dma_start(
                out=a_bias[32 * b:32 * b + NK, :],
                in_=a_neg_v[:, :, b, :],
            )

        # sliver static tiles: rows 0..9 diffs (GpSimd, full-width layout
        # col = 256h + j), rows 32..44 constant A (bf16, 32-aligned partition
        # base for the DVE copies), rows 10..31 zeroed (stationary is 0 there).
        a_bc = a_bf[:].unsqueeze(1).broadcast_to((NK, 2, B))
        d5s = []
        for b in range(4):
            d5 = const_pool.tile([45, 512], BF16, tag=f"d5_{b}")
            nc.vector.memset(d5[:], 0.0)
            nc.vector.tensor_copy(
                d5[32:45, :].rearrange("p (h j) -> p h j", h=2), a_bc)
            d5s.append(d5)

        feat_tile = const_pool.tile([128, 64], F32, tag="feat")
        tsum_sb = const_pool.tile([NK, B - BLK], F32, tag="tsum")
        # completion accumulator reuses a phase-A PSUM slot (idle in phase B);
        # cleared by a zero-stationary matmul so any completion order works
        psT = psum_a.tile([NK, B - BLK], F32, tag="pa")
        nc.tensor.matmul(
            psT[0:NK, :],
            s_tile[0:128, 400:400 + NK],
            xt_all[:, 0:B - BLK],
            start=True,
            stop=False,
        )

        # ---- Phase B: pairwise L1 + exp + batch-sum, with symmetry skew ----
        # g order: interleave PE-heavy early blocks with DVE-heavy late blocks
        # to even per-g engine loads; balanced q3 then completion-free q7
        # last so the tsum copy/DMA overlaps the drain.
        g_order = (
            [24, 0, 25, 1, 26, 2, 27, 3]
            + [20, 4, 21, 5, 22, 6, 23, 7]
            + [16, 8, 17, 9, 18, 10, 19, 11]
            + [12, 13, 14, 15]
            + [28, 29, 30, 31]
        )
        for pos, g in enumerate(g_order):  # octet of batch rows: i = 8g+2b+h
            q = g // GPB               # i-block
            jlo = q * BLK
            F = B - jlo                # j-window size per h
            pl1 = psum_b.tile([128, 512], F32, tag="pl1")
            for b in range(4):
                dts = [
                    diff_pool.tile([128, 512], BF16, tag=f"d{t}", name=f"d{t}")
                    for t in range(5)
                ]
                for h in range(2):
                    i = 8 * g + 2 * b + h
                    if (b, h) == (3, 1):
                        # rebalance: one chunk-0 relu per octet on GpSimd
                        nc.gpsimd.tensor_scalar(
                            dts[0][:, F * h:F * (h + 1)],
                            act_bf[0][:, jlo:jlo + F],
                            act_f32[0][:, i:i + 1],
                            0.0,
                            op0=mybir.AluOpType.subtract,
                            op1=mybir.AluOpType.max,
                        )
                    else:
                        nc.scalar.activation(
                            dts[0][:, F * h:F * (h + 1)],
                            act_bf[0][:, jlo:jlo + F],
                            mybir.ActivationFunctionType.Relu,
                            bias=act_neg0[:, i:i + 1],
                            scale=1.0,
                        )
                    for t in range(1, 5):
                        nc.vector.tensor_scalar(
                            dts[t][:, F * h:F * (h + 1)],
                            act_bf[t][:, jlo:jlo + F],
                            act_f32[t][:, i:i + 1],
                            0.0,
                            op0=mybir.AluOpType.subtract,
                            op1=mybir.AluOpType.max,
                        )
                    nc.gpsimd.tensor_scalar(
                        d5s[b][0:10, 256 * h + jlo:256 * h + jlo + F],
                        act_bf[5][:, jlo:jlo + F],
                        act_f32[5][:, i:i + 1],
                        0.0,
                        op0=mybir.AluOpType.subtract,
                        op1=mybir.AluOpType.max,
                    )
                # d-reduction on PE: pl1[32b + r, F*h + jj] = 2*relu_sum - A
                for t in range(5):
                    nc.tensor.matmul(
                        pl1[32 * b:32 * b + 32, 0:2 * F],
                        s_tile[0:128, 32 * t:32 * t + 32],
                        dts[t][:, 0:2 * F],
                        start=(t == 0),
                        stop=False,
                        tile_position=(0, 32 * b),
                    )
                # sliver + A-fold: moving rows 0..22, 3D AP (h, j-window)
                d5m = d5s[b][:].rearrange("p (h j) -> p h j", h=2)[:, :, jlo:jlo + F]
                nc.tensor.matmul(
                    pl1[32 * b:32 * b + NK, 0:2 * F],
                    s_tile[0:45, 160:173],
                    d5m,
                    start=False,
                    stop=True,
                    tile_position=(0, 32 * b),
                )
            for h in range(2):
                col = 2 * g + h
                jt = junk_pool.tile([128, 256], BF16, tag="jt")
                nc.scalar.activation(
                    jt[:, 0:F],
                    pl1[:, F * h:F * (h + 1)],
                    mybir.ActivationFunctionType.Exp,
                    bias=a_bias[:, col:col + 1],
                    scale=-1.0,
                    accum_out=feat_tile[:, col:col + 1],
                )
                if q < NBLK - 1:
                    # completion: psT[r, j - BLK] += sum_{(b,r')} E tile cols
                    # (psT was cleared by the init matmul; accumulate freely)
                    nc.tensor.matmul(
                        psT[0:NK, q * BLK:B - BLK],
                        s_tile[0:128, 384:384 + NK],
                        jt[:, BLK:F],
                        start=False,
                        stop=(g == 15 and h == 1),
                    )
            # finished feature columns: overlap their DMA out
            for c0, c1 in {7: ((0, 8), (48, 56)), 15: ((8, 16), (40, 48)),
                           23: ((16, 24), (32, 40)),
                           27: ((24, 32),), 30: ((56, 62),)}.get(pos, ()):
                nc.sync.dma_start(out=feat_d[:, c0:c1], in_=feat_tile[:, c0:c1])
            if pos == 27:
                # completion accumulator finished: ship it out during block q7
                nc.vector.tensor_copy(tsum_sb[:], psT[:])
                nc.sync.dma_start(out=tsum_d[:], in_=tsum_sb[:])

        nc.sync.dma_start(out=feat_d[:, 62:64], in_=feat_tile[:, 62:64])
    nc.finalize()
    return nc


def _build_s_pack():
    s = np.zeros((128, 416), np.float32)
    # full chunks: Sx2 (cols 32t + r) and S1 (cols 192 + 32t + r)
    q = np.arange(COLS)
    t = q // 128
    p = q % 128
    r = q // DK
    s[p, 32 * t + r] = 2.0
    s[p, 192 + 32 * t + r] = 1.0
    # sliver A-fold rows (at partitions 32..44 of the sliver tile): -1
    for rr in range(NK):
        s[32 + rr, 160 + rr] = -1.0
    # completion stationary: sum over b of partition (b, r') -> row r'
    for b in range(4):
        for rr in range(NK):
            s[32 * b + rr, 384 + rr] = 1.0
    return s.astype(ml_dtypes.bfloat16)


_NC_CACHE = None


def _get_nc():
    global _NC_CACHE
    if _NC_CACHE is None:
        _NC_CACHE = build_nc()
    return _NC_CACHE


def make_in_maps(x, weight):
    x = np.asarray(x, np.float32)
    weight = np.asarray(weight, np.float32)
    xT = np.ascontiguousarray(x.T).astype(ml_dtypes.bfloat16)
    wp = np.zeros((IN_D, COLS * N_CORES), np.float32)
    wp[:, :weight.shape[1]] = weight
    s_pack = _build_s_pack()
    return [
        {
            "xT": xT,
            "w": np.ascontiguousarray(wp[:, COLS * c:COLS * (c + 1)]).astype(
                ml_dtypes.bfloat16),
            "s": s_pack,
        }
        for c in range(N_CORES)
    ]


def assemble(x, results):
    """results: per-core dicts with 'feat' [128, 64] and 'tsum' [13, 192]."""
    x = np.asarray(x, np.float32)
    feats = []
    for c in range(N_CORES):
        f = np.asarray(results[c]["feat"], np.float32)
        ts = np.asarray(results[c]["tsum"], np.float32)   # [13, B - BLK]
        # f[32b + r, 2g + h] = sum over computed j of E for i = 8g+2b+h
        F = f.reshape(4, 32, 32, 2)[:, :NK]        # [b, r, g, h]
        feat = F.transpose(2, 0, 3, 1).reshape(B, NK)
        # completion for i >= BLK: += sum_{i' in lower blocks} E[i', j=i]
        feat[BLK:, :] += ts.T
        feats.append(feat)
    features = np.concatenate(feats, axis=1)[:, :100]
    return np.concatenate([x, features], axis=1)


def kernel(x, weight):
    in_maps = make_in_maps(x, weight)
    nc = _get_nc()
    res = run_bass_kernel_spmd(nc, in_maps, list(range(N_CORES)))
    return assemble(x, res.results)


# revision 50
# speedup vs baseline: 1.0355x; 1.0355x over previous
"""MiniBatchDiscrimination Trainium2 kernel.

Math (per reference):
    act = (x @ W).reshape(B, K, D)              # B=256, K=100, D=50
    l1[i,k,j] = sum_d |act[i,k,d] - act[j,k,d]|
    features[i,k] = sum_j exp(-l1[i,k,j])
    out = concat([x, features], axis=1)

Sharding: kernels K are sharded across the 8 cores -- 13 kernels (650
columns of act_T) per core, K padded 100->104 with zero weight columns.
No collectives; each core handles the full BxB pairwise work for its 13
kernels.

Per-core algorithm (transposed layout, act_T[col, j] with col=(k,d)):
  Phase A: act_T = W_shard.T @ x.T on PE, cast to bf16 (plus an fp32
    upcast used as the tensor_scalar per-partition operand).  A small
    row-sum table A[r, j] = sum_d act[(r,d), j] is also computed on PE,
    and a bias table A_bias[32b+r, 2g+h] = -A[r, 8g+2b+h] is built with
    8 strided SBUF->SBUF DMAs.
  Phase B uses |y| = 2*relu(y) - y, i.e.
    l1[r; i, j] = 2*sum_d relu(act_j - act_i) - A[r, j] + A[r, i].
    - diff rows split 650 = 5*128 + 10 into 5 full chunks + a sliver.
      Per (i, chunk): one op computing relu(act_T[:, j-window] - act_T[:, i]):
      chunk 0 on ScalarE (Relu activation, per-partition bias), chunks
      1-4 on DVE tensor_scalar (bf16 4x mode), sliver on GpSimd.
    - PE d-reduction: block-diagonal 2.0-valued stationary contracts each
      chunk into PSUM; the sliver chunk's tile carries 13 extra constant
      rows holding A (bf16) so the same matmul also subtracts A[r, j]
      (stationary entries -1) -- no separate A-correction matmuls.
    - ScalarE exp: out = Exp(-pl1 + bias) with bias = -A[r, i] from the
      A_bias table, so the result is the true E[i,j] = exp(-l1); the
      accum_out gives sum_j for free.  Diagonal i==j is exactly 1.
  Symmetry skew (E[i,j] = E[j,i] up to fp32 rounding): i-block q of
    NBLK computes only j >= q*BLK.  The skipped contributions
    features[i] += sum_{j < block(i)} E[i,j] = sum_{i' in lower blocks}
    E[i', j=i] are recovered by accumulating PE column-sum matmuls
    (ones stationary over the (b,r) partition packing) of the retained
    exp tiles into a [13, 256-BLK] PSUM tile, shipped out as 'tsum'.
  Host: features = accum feats + tsum completion; concat with x.
"""

import numpy as np
import ml_dtypes
from contextlib import ExitStack

import concourse.bass as bass
import concourse.bacc as bacc
import concourse.tile as tile
from concourse import mybir
from concourse.bass_utils import run_bass_kernel_spmd

B = 256          # batch
IN_D = 1024      # input dim
NK = 13          # kernels per core (8*13 = 104 >= 100)
DK = 50          # dim per kernel
COLS = NK * DK   # 650 act_T rows per core
N_CORES = 8
NBLK = 8         # symmetry skew blocks over the batch
BLK = B // NBLK  # i/j block size
GPB = 32 // NBLK     # g-octets per block
FULL = [(0, 128), (128, 128), (256, 128), (384, 128), (512, 128)]
SLIV = (640, 10)

F32 = mybir.dt.float32
BF16 = mybir.dt.bfloat16


def build_nc():
    nc = bacc.Bacc()
    xT_d = nc.declare_dram_parameter("xT", [IN_D, B], BF16, isOutput=False)
    w_d = nc.declare_dram_parameter("w", [IN_D, COLS], BF16, isOutput=False)
    s_d = nc.declare_dram_parameter("s", [128, 416], BF16, isOutput=False)
    feat_d = nc.declare_dram_parameter("feat", [128, 64], F32, isOutput=True)
    tsum_d = nc.declare_dram_parameter("tsum", [NK, B - BLK], F32, isOutput=True)

    with ExitStack() as ctx:
        tc = ctx.enter_context(tile.TileContext(nc))
        const_pool = ctx.enter_context(tc.tile_pool(name="const", bufs=1))
        psum_a = ctx.enter_context(tc.tile_pool(name="psum_a", bufs=2, space="PSUM"))
        psum_b = ctx.enter_context(tc.tile_pool(name="psum_b", bufs=4, space="PSUM"))
        diff_pool = ctx.enter_context(tc.tile_pool(name="diff", bufs=8))
        junk_pool = ctx.enter_context(tc.tile_pool(name="junk", bufs=4))

        # ---- load inputs (batched DMAs, ordered by first use) ----
        s_tile = const_pool.tile([128, 416], BF16, tag="s")
        nc.sync.dma_start(out=s_tile[:], in_=s_d[:])
        xt_all = const_pool.tile([128, 8 * B], BF16, tag="xt")
        nc.sync.dma_start(
            out=xt_all[:].rearrange("p (k j) -> p k j", k=8),
            in_=xT_d[:].rearrange("(k p) j -> p k j", k=8),
        )
        # w loaded in 3 slices so early chunks' matmuls start before the
        # whole tensor lands (512B+ descriptor runs keep DMA cost low)
        w_all = const_pool.tile([128, 8 * COLS], BF16, tag="w")
        w_view = w_all[:].rearrange("p (k c) -> p k c", k=8)
        for c0, c1 in ((0, 256), (256, 512), (512, 650)):
            nc.sync.dma_start(
                out=w_view[:, :, c0:c1],
                in_=w_d[:, c0:c1].rearrange("(k p) c -> p k c", k=8),
            )

        # ---- PE warm-up during the DMA wait (p-state ramp to full clock) ----
        wu = const_pool.tile([128, 640], BF16, tag="wu")
        nc.vector.memset(wu[:], 0.0)
        pwu = psum_b.tile([128, 512], F32, tag="pl1")
        for _ in range(8):
            nc.tensor.matmul(
                pwu[:], wu[:, 512:640], wu[:, 0:512], start=True, stop=True,
            )

        # ---- Phase A: act_T = W.T @ xT  (per chunk of act_T rows) ----
        CHUNKS = FULL + [SLIV]
        act_bf = []   # bf16 streaming operand
        act_f32 = []  # fp32 upcast (tensor_scalar per-partition operand)
        for t, (mstart, msz) in enumerate(CHUNKS):
            pa = psum_a.tile([msz, B], F32, tag="pa")
            for k in range(8):
                nc.tensor.matmul(
                    pa[:],
                    w_all[:, COLS * k + mstart:COLS * k + mstart + msz],
                    xt_all[:, B * k:B * (k + 1)],
                    start=(k == 0),
                    stop=(k == 7),
                )
            tb = const_pool.tile([msz, B], BF16, tag=f"actb{t}")
            nc.vector.tensor_copy(tb[:], pa[:])
            act_bf.append(tb)
            if t == 0:
                # chunk 0 mostly runs on ScalarE (Relu bias): negation needed,
                # plus the fp32 upcast for the GpSimd-assigned (b,h) slot
                tn = const_pool.tile([msz, B], F32, tag="actn0")
                nc.gpsimd.tensor_scalar_mul(tn[:], tb[:], -1.0)
                act_neg0 = tn
                tf = const_pool.tile([msz, B], F32, tag="actf0")
                nc.gpsimd.tensor_copy(tf[:], tb[:])
                act_f32.append(tf)
            else:
                tf = const_pool.tile([msz, B], F32, tag=f"actf{t}")
                nc.gpsimd.tensor_copy(tf[:], tb[:])
                act_f32.append(tf)

        # A[r, j] = sum_d act[(r,d), j]  (exact fp32 accumulation of bf16)
        pA = psum_a.tile([32, B], F32, tag="pa")
        for t, (mstart, msz) in enumerate(CHUNKS):
            nc.tensor.matmul(
                pA[:],
                s_tile[0:msz, 192 + 32 * t:192 + 32 * t + 32],
                act_bf[t][:],
                start=(t == 0),
                stop=(t == len(CHUNKS) - 1),
            )
        # A-table prep runs on ScalarE (idle during phase A) so DVE/GpSimd
        # program order stays free to start phase-B diffs chunk by chunk.
        a_bf = const_pool.tile([NK, B], BF16, tag="a_bf")
        nc.scalar.copy(a_bf[:], pA[0:NK, :])
        a_neg = const_pool.tile([NK, B], F32, tag="a_neg")
        nc.scalar.mul(a_neg[:], a_bf[:], -1.0)

        # bias table: A_bias[32b + r, 2g + h] = -A[r, 8g + 2b + h]
        # (zeroed first: rows 32b+13..32b+31 feed exp on don't-care partitions
        # and must stay finite for the completion matmul's 0-weight contract)
        a_bias = const_pool.tile([128, 64], F32, tag="a_bias")
        nc.vector.memset(a_bias[:], 0.0)
        a_neg_v = a_neg[:].rearrange("r (g b h) -> r g b h", b=4, h=2)
        for b in range(4):
            # [13 parts, (g,h)=64] per b-group (HWDGE is idle by this point)
            nc.sync.dma_start(
                out=a_bias[32 * b:32 * b + NK, :],
                in_=a_neg_v[:, :, b, :],
            )

        # sliver static tiles: rows 0..9 diffs (GpSimd, full-width layout
        # col = 256h + j), rows 32..44 constant A (bf16, 32-aligned partition
        # base for the DVE copies), rows 10..31 zeroed (stationary is 0 there).
        a_bc = a_bf[:].unsqueeze(1).broadcast_to((NK, 2, B))
        d5s = []
        for b in range(4):
            d5 = const_pool.tile([45, 512], BF16, tag=f"d5_{b}")
            nc.vector.memset(d5[:], 0.0)
            nc.vector.tensor_copy(
                d5[32:45, :].rearrange("p (h j) -> p h j", h=2), a_bc)
            d5s.append(d5)

        feat_tile = const_pool.tile([128, 64], F32, tag="feat")
        tsum_sb = const_pool.tile([NK, B - BLK], F32, tag="tsum")
        # completion accumulator reuses a phase-A PSUM slot (idle in phase B);
        # cleared by a zero-stationary matmul so any completion order works
        psT = psum_a.tile([NK, B - BLK], F32, tag="pa")
        nc.tensor.matmul(
            psT[0:NK, :],
            s_tile[0:128, 400:400 + NK],
            xt_all[:, 0:B - BLK],
            start=True,
            stop=False,
        )

        # ---- Phase B: pairwise L1 + exp + batch-sum, with symmetry skew ----
        # g order: interleave PE-heavy early blocks with DVE-heavy late blocks
        # to even per-g engine loads; balanced q3 then completion-free q7
        # last so the tsum copy/DMA overlaps the drain.
        g_order = (
            [24, 0, 25, 1, 26, 2, 27, 3]
            + [20, 4, 21, 5, 22, 6, 23, 7]
            + [16, 8, 17, 9, 18, 10, 19, 11]
            + [12, 13, 14, 15]
            + [28, 29, 30, 31]
        )
        for pos, g in enumerate(g_order):  # octet of batch rows: i = 8g+2b+h
            q = g // GPB               # i-block
            jlo = q * BLK
            F = B - jlo                # j-window size per h
            pl1 = psum_b.tile([128, 512], F32, tag="pl1")
            for b in range(4):
                dts = [
                    diff_pool.tile([128, 512], BF16, tag=f"d{t}", name=f"d{t}")
                    for t in range(5)
                ]
                for h in range(2):
                    i = 8 * g + 2 * b + h
                    if (b, h) == (3, 1):
                        # rebalance: one chunk-0 relu per octet on GpSimd
                        nc.gpsimd.tensor_scalar(
                            dts[0][:, F * h:F * (h + 1)],
                            act_bf[0][:, jlo:jlo + F],
                            act_f32[0][:, i:i + 1],
                            0.0,
                            op0=mybir.AluOpType.subtract,
                            op1=mybir.AluOpType.max,
                        )
                    else:
                        nc.scalar.activation(
                            dts[0][:, F * h:F * (h + 1)],
                            act_bf[0][:, jlo:jlo + F],
                            mybir.ActivationFunctionType.Relu,
                            bias=act_neg0[:, i:i + 1],
                            scale=1.0,
                        )
                    for t in range(1, 5):
                        nc.vector.tensor_scalar(
                            dts[t][:, F * h:F * (h + 1)],
                            act_bf[t][:, jlo:jlo + F],
                            act_f32[t][:, i:i + 1],
                            0.0,
                            op0=mybir.AluOpType.subtract,
                            op1=mybir.AluOpType.max,
                        )
                    nc.gpsimd.tensor_scalar(
                        d5s[b][0:10, 256 * h + jlo:256 * h + jlo + F],
                        act_bf[5][:, jlo:jlo + F],
                        act_f32[5][:, i:i + 1],
                        0.0,
                        op0=mybir.AluOpType.subtract,
                        op1=mybir.AluOpType.max,
                    )
                # d-reduction on PE: pl1[32b + r, F*h + jj] = 2*relu_sum - A
                for t in range(5):
                    nc.tensor.matmul(
                        pl1[32 * b:32 * b + 32, 0:2 * F],
                        s_tile[0:128, 32 * t:32 * t + 32],
                        dts[t][:, 0:2 * F],
                        start=(t == 0),
                        stop=False,
                        tile_position=(0, 32 * b),
                    )
                # sliver + A-fold: moving rows 0..22, 3D AP (h, j-window)
                d5m = d5s[b][:].rearrange("p (h j) -> p h j", h=2)[:, :, jlo:jlo + F]
                nc.tensor.matmul(
                    pl1[32 * b:32 * b + NK, 0:2 * F],
                    s_tile[0:45, 160:173],
                    d5m,
                    start=False,
                    stop=True,
                    tile_position=(0, 32 * b),
                )
            for h in range(2):
                col = 2 * g + h
                jt = junk_pool.tile([128, 256], BF16, tag="jt")
                nc.scalar.activation(
                    jt[:, 0:F],
                    pl1[:, F * h:F * (h + 1)],
                    mybir.ActivationFunctionType.Exp,
                    bias=a_bias[:, col:col + 1],
                    scale=-1.0,
                    accum_out=feat_tile[:, col:col + 1],
                )
                if q < NBLK - 1:
                    # completion: psT[r, j - BLK] += sum_{(b,r')} E tile cols
                    # (psT was cleared by the init matmul; accumulate freely)
                    nc.tensor.matmul(
                        psT[0:NK, q * BLK:B - BLK],
                        s_tile[0:128, 384:384 + NK],
                        jt[:, BLK:F],
                        start=False,
                        stop=(g == 15 and h == 1),
                    )
            # finished feature columns: overlap their DMA out
            for c0, c1 in {7: ((0, 8), (48, 56)), 15: ((8, 16), (40, 48)),
                           23: ((16, 24), (32, 40)),
                           27: ((24, 32),), 30: ((56, 62),)}.get(pos, ()):
                nc.sync.dma_start(out=feat_d[:, c0:c1], in_=feat_tile[:, c0:c1])
            if pos == 27:
                # completion accumulator finished: ship it out during block q7
                nc.vector.tensor_copy(tsum_sb[:], psT[:])
                nc.sync.dma_start(out=tsum_d[:], in_=tsum_sb[:])

        nc.sync.dma_start(out=feat_d[:, 62:64], in_=feat_tile[:, 62:64])
    nc.finalize()
    return nc


def _build_s_pack():
    s = np.zeros((128, 416), np.float32)
    # full chunks: Sx2 (cols 32t + r) and S1 (cols 192 + 32t + r)
    q = np.arange(COLS)
    t = q // 128
    p = q % 128
    r = q // DK
    s[p, 32 * t + r] = 2.0
    s[p, 192 + 32 * t + r] = 1.0
    # sliver A-fold rows (at partitions 32..44 of the sliver tile): -1
    for rr in range(NK):
        s[32 + rr, 160 + rr] = -1.0
    # completion stationary: sum over b of partition (b, r') -> row r'
    for b in range(4):
        for rr in range(NK):
            s[32 * b + rr, 384 + rr] = 1.0
    return s.astype(ml_dtypes.bfloat16)


_NC_CACHE = None


def _get_nc():
    global _NC_CACHE
    if _NC_CACHE is None:
        _NC_CACHE = build_nc()
    return _NC_CACHE


def make_in_maps(x, weight):
    x = np.asarray(x, np.float32)
    weight = np.asarray(weight, np.float32)
    xT = np.ascontiguousarray(x.T).astype(ml_dtypes.bfloat16)
    wp = np.zeros((IN_D, COLS * N_CORES), np.float32)
    wp[:, :weight.shape[1]] = weight
    s_pack = _build_s_pack()
    return [
        {
            "xT": xT,
            "w": np.ascontiguousarray(wp[:, COLS * c:COLS * (c + 1)]).astype(
                ml_dtypes.bfloat16),
            "s": s_pack,
        }
        for c in range(N_CORES)
    ]


def assemble(x, results):
    """results: per-core dicts with 'feat' [128, 64] and 'tsum' [13, 192]."""
    x = np.asarray(x, np.float32)
    feats = []
    for c in range(N_CORES):
        f = np.asarray(results[c]["feat"], np.float32)
        ts = np.asarray(results[c]["tsum"], np.float32)   # [13, B - BLK]
        # f[32b + r, 2g + h] = sum over computed j of E for i = 8g+2b+h
        F = f.reshape(4, 32, 32, 2)[:, :NK]        # [b, r, g, h]
        feat = F.transpose(2, 0, 3, 1).reshape(B, NK)
        # completion for i >= BLK: += sum_{i' in lower blocks} E[i', j=i]
        feat[BLK:, :] += ts.T
        feats.append(feat)
    features = np.concatenate(feats, axis=1)[:, :100]
    return np.concatenate([x, features], axis=1)


def kernel(x, weight):
    in_maps = make_in_maps(x, weight)
    nc = _get_nc()
    res = run_bass_kernel_spmd(nc, in_maps, list(range(N_CORES)))
    return assemble(x, res.results)
